# revision 14
# baseline (speedup 1.0000x reference)
"""MoE FFN (grouped top-1 routing, SwiGLU experts) on 8 Trainium2 NeuronCores.

Strategy (expert-parallel with static 2-segment load balancing):
  - Host computes the (tiny) routers in float64: sigmoid(x @ macro_w) -> top-1
    group of 4; within the selected group both 2 experts are active
    (TOP_K == EXPERTS_PER_GROUP) with sigmoid-normalized weights.
  - Each core processes C = s0 + s1 token columns in two statically-sized
    segments.  Each segment has its own full SwiGLU weight-set input, so the
    host can assign ANY expert to any (core, segment) bucket.  A small search
    picks (s0, s1) so the 8 expert token-loads pack into the 16 buckets with
    minimal C (544 vs 608 for naive per-expert capacity on the benchmark
    routing distribution).
  - The per-token routing weight is applied on the HOST to the fp32 partial
    outputs (y[t] = w0*y_e0[t] + w1*y_e1[t]), so the device never sees it and
    x is shipped only once.
  - Weights are pre-interleaved on the host into partition-major tensors
    ([P, FO, {gate,up}, DO, P]) so every DMA block is one contiguous run per
    partition AND delivers gate+up for an f-tile together, in exact PE
    consumption order on a single HWDGE ring.  Segment A runs fully
    (gate/up then down), then segment B, halving the early weight-demand
    rate vs the ~358 GB/s per-core HBM cap.
  - Device kernel per segment: Y^T = down^T @ (silu(gate^T X^T) * (up^T X^T)),
    features on SBUF partitions, tokens on the free dim, bf16 storage/matmuls,
    fp32 PSUM accumulation.  Dummy matmuls on a zeroed tile during the
    initial DMA wait warm the PE clock-gate (HAM) to 2.4 GHz for free.
"""

import numpy as np
import ml_dtypes

import concourse.bass as bass  # noqa: F401  (bass types via bacc)
import concourse.mybir as mybir
import concourse.tile as tile
from concourse import bacc
from concourse.bass_utils import run_bass_kernel_spmd

P = 128
D_MODEL = 1024
FFN_DIM = 2048
NUM_EXPERTS = 8
NUM_GROUPS = 4
EPS = 1e-9
DO = D_MODEL // P   # 8 k-tiles over D
FO = FFN_DIM // P   # 16 f-tiles over F
ND = 2              # d-tiles per down-weight block

F32 = mybir.dt.float32
BF16 = mybir.dt.bfloat16

N_CORES = 8

# gate/up f-blocks (start_fo, n_fo): segment A leads with 1-fo blocks so the
# PE starts as soon as possible; later/segment-B blocks are wide.
BLOCKS_A = [(0, 1), (1, 1), (2, 2), (4, 2), (6, 2), (8, 4), (12, 4)]
BLOCKS_B = [(0, 4), (4, 4), (8, 4), (12, 4)]

_BUILD_CACHE: dict[tuple, object] = {}
_PLAN_CACHE: dict[tuple, tuple] = {}
LAST_RESULTS = None  # stashed BassKernelResults for test harnesses


# ──────────────────────────────────────────────────────────────────────
# Device program
# ──────────────────────────────────────────────────────────────────────

def _build(s0: int, s1: int):
    """Bass/Tile program: C=s0+s1 token columns, two segments, each with its
    own full SwiGLU weight set in host-pre-interleaved layout."""
    C = s0 + s1
    seg_len = [s0, s1]

    nc = bacc.Bacc(
        "TRN2",
        target_bir_lowering=False,
        debug=False,
        enable_asserts=False,
        num_devices=N_CORES,
    )
    xts_d = [
        nc.dram_tensor("xta", [P, DO, s0], BF16, kind="ExternalInput").ap(),
        nc.dram_tensor("xtb", [P, DO, s1], BF16, kind="ExternalInput").ap(),
    ]
    gus_d, dws_d = [], []
    for k in range(2):
        gus_d.append(nc.dram_tensor(f"gu{k}", [P, FO * 2 * DO, P], BF16,
                                    kind="ExternalInput").ap())
        dws_d.append(nc.dram_tensor(f"dwb{k}", [P, (DO // ND) * FO, ND * P],
                                    BF16, kind="ExternalInput").ap())
    yt = nc.dram_tensor("yt", [D_MODEL, C], F32, kind="ExternalOutput").ap()

    with tile.TileContext(nc) as tc:
        with (
            tc.tile_pool(name="xp", bufs=1) as xp,
            tc.tile_pool(name="wp", bufs=4) as wp,
            tc.tile_pool(name="dp", bufs=4) as dp,
            tc.tile_pool(name="hp", bufs=1) as hp,
            tc.tile_pool(name="sp", bufs=4) as sp,
            tc.tile_pool(name="yp", bufs=4) as yp,
            tc.tile_pool(name="pgu", bufs=6, space="PSUM") as pgu,
            tc.tile_pool(name="pd", bufs=2, space="PSUM") as pd,
        ):
            # HAM warm-up: dummy matmuls on a zeroed tile keep the PE busy
            # during the initial DMA wait so the clock gate is at 2.4 GHz
            # when real work arrives.
            zt = xp.tile([P, 640], BF16, tag="z")
            nc.gpsimd.memset(zt[:], 0.0)
            pz = pgu.tile([P, 512], F32, tag="ps", name="warm")
            for _ in range(12):
                nc.tensor.matmul(pz[:], zt[:, 0:128], zt[:, 128:640],
                                 start=True, stop=True)

            xa = xp.tile([P, DO, s0], BF16, tag="xa")
            xb = xp.tile([P, DO, s1], BF16, tag="xb")
            xseg = [xa, xb]
            gus, dts = {}, {}

            def load_gu(k, blocks):
                for b, (sfo, nfo) in enumerate(blocks):
                    g = wp.tile([P, nfo * 2 * DO, P], BF16, tag="gu",
                                name=f"gu{k}_{b}")
                    nc.sync.dma_start(
                        g[:], gus_d[k][:, sfo * 2 * DO:(sfo + nfo) * 2 * DO])
                    gus[(k, b)] = g

            def load_dw(k):
                for db in range(DO // ND):
                    dt_ = dp.tile([P, FO, ND * P], BF16, tag="dt",
                                  name=f"d{k}_{db}")
                    nc.sync.dma_start(
                        dt_[:], dws_d[k][:, db * FO:(db + 1) * FO])
                    dts[(k, db)] = dt_

            # ALL loads on the single sync HWDGE ring in strict consumption
            # order.  Critical prefix: gate half of fo0 + xA (0.87 MB); the
            # up half of fo0 follows xA and lands before the gate group ends.
            g00a = wp.tile([P, DO, P], BF16, tag="gu", name="gu0_0g")
            nc.sync.dma_start(g00a[:], gus_d[0][:, 0:DO])
            nc.sync.dma_start(xa[:], xts_d[0][:])
            g00b = wp.tile([P, DO, P], BF16, tag="gu", name="gu0_0u")
            nc.sync.dma_start(g00b[:], gus_d[0][:, DO:2 * DO])
            gus[(0, 0)] = (g00a, g00b)
            for b, (sfo, nfo) in enumerate(BLOCKS_A):
                if b == 0:
                    continue
                g = wp.tile([P, nfo * 2 * DO, P], BF16, tag="gu",
                            name=f"gu0_{b}")
                nc.sync.dma_start(
                    g[:], gus_d[0][:, sfo * 2 * DO:(sfo + nfo) * 2 * DO])
                gus[(0, b)] = g
            load_dw(0)
            nc.sync.dma_start(xb[:], xts_d[1][:])
            load_gu(1, BLOCKS_B)
            load_dw(1)

            hs = hp.tile([P, FO, C], BF16, tag="h")

            for si in range(2):
                k = si
                slen = seg_len[si]
                off = 0 if si == 0 else s0
                csl = slice(off, off + slen)
                xs = xseg[si]
                blocks = BLOCKS_A if si == 0 else BLOCKS_B
                fo2blk = {}
                for b, (sfo, nfo) in enumerate(blocks):
                    for fo in range(sfo, sfo + nfo):
                        fo2blk[fo] = b

                # gate/up for this segment
                for fo in range(FO):
                    b = fo2blk[fo]
                    fl = fo - blocks[b][0]
                    gu4 = gus[(k, b)]
                    if isinstance(gu4, tuple):
                        gsel = lambda kind, do: gu4[kind][:, do, :]
                    else:
                        gsel = lambda kind, do: gu4[:, (fl * 2 + kind) * DO + do, :]
                    psg = pgu.tile([P, slen], F32, tag="ps", name=f"psg_{fo}_{si}")
                    psu = pgu.tile([P, slen], F32, tag="ps", name=f"psu_{fo}_{si}")
                    for do in range(DO):
                        nc.tensor.matmul(
                            psg[:], gsel(0, do), xs[:, do, :],
                            start=(do == 0), stop=(do == DO - 1),
                        )
                    for do in range(DO):
                        nc.tensor.matmul(
                            psu[:], gsel(1, do), xs[:, do, :],
                            start=(do == 0), stop=(do == DO - 1),
                        )
                    sg = sp.tile([P, slen], F32, tag="sg")
                    nc.scalar.activation(
                        sg[:], psg[:], mybir.ActivationFunctionType.Silu
                    )
                    nc.vector.tensor_mul(out=hs[:, fo, csl], in0=sg[:], in1=psu[:])

                # down for this segment
                for db in range(DO // ND):
                    for half in range(ND):
                        do = db * ND + half
                        dsl = slice(half * P, (half + 1) * P)
                        psy = pd.tile([P, slen], F32, tag="psy",
                                      name=f"psy_{do}_{si}")
                        for fo in range(FO):
                            nc.tensor.matmul(
                                psy[:], dts[(k, db)][:, fo, dsl], hs[:, fo, csl],
                                start=(fo == 0), stop=(fo == FO - 1),
                            )
                        yo = yp.tile([P, slen], F32, tag="yo")
                        nc.any.tensor_copy(out=yo[:], in_=psy[:])
                        nc.scalar.dma_start(yt[do * P:(do + 1) * P, csl], yo[:])
    nc.finalize()
    return nc


def _get_program(s0: int, s1: int):
    key = (s0, s1)
    if key not in _BUILD_CACHE:
        _BUILD_CACHE[key] = _build(s0, s1)
    return _BUILD_CACHE[key]


# ──────────────────────────────────────────────────────────────────────
# Host routing
# ──────────────────────────────────────────────────────────────────────

def _sigmoid(z):
    return 1.0 / (1.0 + np.exp(-z))


def _route(xf32, macro_w, micro_w):
    """Host routers in float64. Returns group index per token and per-token
    weights for the 2 experts of the selected group (float32)."""
    xf = xf32.astype(np.float64)
    ms = _sigmoid(xf @ macro_w.astype(np.float64))  # [T, G]
    g_sel = np.argmax(ms, axis=1)
    T = xf.shape[0]
    mval = ms[np.arange(T), g_sel]
    mv = mval / (mval + EPS)

    w2 = np.zeros((T, 2), np.float64)
    for g in range(NUM_GROUPS):
        idx = np.nonzero(g_sel == g)[0]
        if idx.size == 0:
            continue
        s = _sigmoid(xf[idx] @ micro_w[g].astype(np.float64))  # [n, 2]
        denom = s[:, 0] + s[:, 1] + EPS
        w2[idx, 0] = mv[idx] * s[:, 0] / denom
        w2[idx, 1] = mv[idx] * s[:, 1] / denom
    return g_sel, w2.astype(np.float32)


# ──────────────────────────────────────────────────────────────────────
# Segment-size search + bucket assignment
# ──────────────────────────────────────────────────────────────────────

def _feasible(n_e, s0, s1):
    """Can loads n_e pack into 8 buckets of s0 and 8 of s1 (each bucket a
    single expert)?  Returns per-expert (a, b) bucket counts or None."""
    cands = []
    for n in n_e:
        cc = []
        if n == 0:
            cc.append((0, 0))
        else:
            for a in range(9):
                rem = n - a * s0
                b = 0 if rem <= 0 else -(-rem // s1)
                if b <= 8:
                    cc.append((a, b))
            cc.sort()
            pruned, best_b = [], 99
            for a, b in cc:
                if b < best_b:
                    pruned.append((a, b))
                    best_b = b
            cc = pruned
        cands.append(cc)
    states = {(0, 0): []}
    for cc in cands:
        nxt = {}
        for (ua, ub), hist in states.items():
            for a, b in cc:
                na, nb = ua + a, ub + b
                if na <= 8 and nb <= 8 and (na, nb) not in nxt:
                    nxt[(na, nb)] = hist + [(a, b)]
        states = nxt
        if not states:
            return None
    return next(iter(states.values()))


def _plan(n_e):
    """Pick (s0, s1, ab) minimizing C = s0 + s1."""
    key = tuple(n_e)
    if key in _PLAN_CACHE:
        return _PLAN_CACHE[key]
    best = None
    for s0 in range(64, 513, 16):
        for s1 in range(48, s0 + 1, 16):
            ab = _feasible(n_e, s0, s1)
            if ab is None:
                continue
            c = s0 + s1
            if best is None or c < best[0] or (c == best[0] and s0 < best[1]):
                best = (c, s0, s1, ab)
    if best is None:
        raise RuntimeError(f"no feasible segment plan for loads {n_e}")
    _, s0, s1, ab = best
    plan = (s0, s1, ab)
    _PLAN_CACHE[key] = plan
    return plan


# ──────────────────────────────────────────────────────────────────────
# Entry point
# ──────────────────────────────────────────────────────────────────────

def kernel(x, macro_w, micro_w, gate_w, up_w, down_w):
    global LAST_RESULTS
    x = np.asarray(x)
    B, S, D = x.shape
    T = B * S
    xf = np.ascontiguousarray(x.reshape(T, D).astype(np.float32, copy=False))

    g_sel, w2 = _route(xf, np.asarray(macro_w), np.asarray(micro_w))
    idx_by_g = [np.nonzero(g_sel == g)[0] for g in range(NUM_GROUPS)]
    n_e = [idx_by_g[e // 2].size for e in range(NUM_EXPERTS)]

    s0, s1, ab = _plan(n_e)
    segs = [(0, s0), (s0, s1)]
    nc = _get_program(s0, s1)

    gate_b = np.asarray(gate_w, np.float32).astype(ml_dtypes.bfloat16)
    up_b = np.asarray(up_w, np.float32).astype(ml_dtypes.bfloat16)
    down_b = np.asarray(down_w, np.float32).astype(ml_dtypes.bfloat16)

    # Pre-interleave weights per expert into the device DMA layouts:
    #   gu[e]  : [P, FO, 2, DO, P]   (gate,up interleaved per f-tile)
    #   dwb[e] : [P, DO//ND, FO, ND*P]
    gu_int, dw_int = [], []
    for e in range(NUM_EXPERTS):
        g5 = gate_b[e].reshape(DO, P, FO, P).transpose(1, 2, 0, 3)
        u5 = up_b[e].reshape(DO, P, FO, P).transpose(1, 2, 0, 3)
        gu_int.append(np.ascontiguousarray(
            np.stack([g5, u5], axis=2)).reshape(P, FO * 2 * DO, P))
        dw_int.append(np.ascontiguousarray(
            down_b[e].reshape(FO, P, DO // ND, ND * P).transpose(1, 2, 0, 3)
        ).reshape(P, (DO // ND) * FO, ND * P))

    # hand out buckets: free lists of (core, seg)
    free = [[(c, si) for c in range(N_CORES)] for si in range(2)]
    jobs = {}
    for e in range(NUM_EXPERTS):
        a, b = ab[e]
        ix = idx_by_g[e // 2]
        pos = 0
        for si, cnt in ((0, a), (1, b)):
            cap = (s0, s1)[si]
            for _ in range(cnt):
                c, _si = free[si].pop(0)
                take = ix[pos:pos + cap]
                pos += cap
                jobs[(c, si)] = (e, take)
    for si in range(2):
        for c, _ in free[si]:
            jobs[(c, si)] = (0, np.empty(0, np.int64))

    xfb = xf.astype(ml_dtypes.bfloat16)
    in_maps = []
    for c in range(N_CORES):
        m = {}
        for si, (off, slen) in enumerate(segs):
            e, ix = jobs[(c, si)]
            xt = np.zeros((P, DO, slen), ml_dtypes.bfloat16)
            if ix.size:
                # [n, D] -> [D, n] -> [DO, P, n] -> [P, DO, n]
                xt[:, :, :ix.size] = (
                    xfb[ix].T.reshape(DO, P, ix.size).transpose(1, 0, 2))
            m["xta" if si == 0 else "xtb"] = xt
            m[f"gu{si}"] = gu_int[e]
            m[f"dwb{si}"] = dw_int[e]
        in_maps.append(m)

    res = run_bass_kernel_spmd(nc, in_maps, core_ids=list(range(N_CORES)))
    LAST_RESULTS = res

    y = np.zeros((T, D), np.float32)
    for c in range(N_CORES):
        ytc = res.results[c]["yt"]
        for si, (off, slen) in enumerate(segs):
            e, ix = jobs[(c, si)]
            if ix.size:
                y[ix] += w2[ix, e % 2][:, None] * ytc[:, off:off + ix.size].T
    return y.reshape(B, S, D)


# revision 16
# speedup vs baseline: 1.0014x; 1.0014x over previous
"""MoE FFN (grouped top-1 routing, SwiGLU experts) on 8 Trainium2 NeuronCores.

Strategy (expert-parallel with static 2-segment load balancing):
  - Host computes the (tiny) routers in float64: sigmoid(x @ macro_w) -> top-1
    group of 4; within the selected group both 2 experts are active
    (TOP_K == EXPERTS_PER_GROUP) with sigmoid-normalized weights.
  - Each core processes C = s0 + s1 token columns in two statically-sized
    segments.  Each segment has its own full SwiGLU weight-set input, so the
    host can assign ANY expert to any (core, segment) bucket.  A small search
    picks (s0, s1) so the 8 expert token-loads pack into the 16 buckets with
    minimal C (544 vs 608 for naive per-expert capacity on the benchmark
    routing distribution).
  - The per-token routing weight is applied on the HOST to the fp32 partial
    outputs (y[t] = w0*y_e0[t] + w1*y_e1[t]), so the device never sees it and
    x is shipped only once.
  - Weights are pre-interleaved on the host into partition-major tensors
    ([P, FO, {gate,up}, DO, P]) so every DMA block is one contiguous run per
    partition AND delivers gate+up for an f-tile together, in exact PE
    consumption order on a single HWDGE ring.  Segment A runs fully
    (gate/up then down), then segment B, halving the early weight-demand
    rate vs the ~358 GB/s per-core HBM cap.
  - Device kernel per segment: Y^T = down^T @ (silu(gate^T X^T) * (up^T X^T)),
    features on SBUF partitions, tokens on the free dim, bf16 storage/matmuls,
    fp32 PSUM accumulation.  Dummy matmuls on a zeroed tile during the
    initial DMA wait warm the PE clock-gate (HAM) to 2.4 GHz for free.
"""

import numpy as np
import ml_dtypes

import concourse.bass as bass  # noqa: F401  (bass types via bacc)
import concourse.mybir as mybir
import concourse.tile as tile
from concourse import bacc
from concourse.bass_utils import run_bass_kernel_spmd

P = 128
D_MODEL = 1024
FFN_DIM = 2048
NUM_EXPERTS = 8
NUM_GROUPS = 4
EPS = 1e-9
DO = D_MODEL // P   # 8 k-tiles over D
FO = FFN_DIM // P   # 16 f-tiles over F
ND = 2              # d-tiles per down-weight block

F32 = mybir.dt.float32
BF16 = mybir.dt.bfloat16

N_CORES = 8

# gate/up f-blocks (start_fo, n_fo): segment A leads with 1-fo blocks so the
# PE starts as soon as possible; later/segment-B blocks are wide.
BLOCKS_A = [(0, 1), (1, 1), (2, 2), (4, 2), (6, 2), (8, 4), (12, 4)]
BLOCKS_B = [(0, 4), (4, 4), (8, 4), (12, 4)]

_BUILD_CACHE: dict[tuple, object] = {}
_PLAN_CACHE: dict[tuple, tuple] = {}
LAST_RESULTS = None  # stashed BassKernelResults for test harnesses


# ──────────────────────────────────────────────────────────────────────
# Device program
# ──────────────────────────────────────────────────────────────────────

def _build(s0: int, s1: int):
    """Bass/Tile program: C=s0+s1 token columns, two segments, each with its
    own full SwiGLU weight set in host-pre-interleaved layout."""
    C = s0 + s1
    seg_len = [s0, s1]

    nc = bacc.Bacc(
        "TRN2",
        target_bir_lowering=False,
        debug=False,
        enable_asserts=False,
        num_devices=N_CORES,
    )
    xts_d = [
        nc.dram_tensor("xta", [P, DO, s0], BF16, kind="ExternalInput").ap(),
        nc.dram_tensor("xtb", [P, DO, s1], BF16, kind="ExternalInput").ap(),
    ]
    gus_d, dws_d = [], []
    for k in range(2):
        gus_d.append(nc.dram_tensor(f"gu{k}", [P, FO * 2 * DO, P], BF16,
                                    kind="ExternalInput").ap())
        dws_d.append(nc.dram_tensor(f"dwb{k}", [P, (DO // ND) * FO, ND * P],
                                    BF16, kind="ExternalInput").ap())
    yt = nc.dram_tensor("yt", [D_MODEL, C], F32, kind="ExternalOutput").ap()

    with tile.TileContext(nc) as tc:
        with (
            tc.tile_pool(name="xp", bufs=1) as xp,
            tc.tile_pool(name="wp", bufs=4) as wp,
            tc.tile_pool(name="dp", bufs=4) as dp,
            tc.tile_pool(name="hp", bufs=1) as hp,
            tc.tile_pool(name="sp", bufs=4) as sp,
            tc.tile_pool(name="yp", bufs=4) as yp,
            tc.tile_pool(name="pgu", bufs=6, space="PSUM") as pgu,
            tc.tile_pool(name="pd", bufs=2, space="PSUM") as pd,
        ):
            # HAM warm-up: dummy matmuls on a zeroed tile keep the PE busy
            # during the initial DMA wait so the clock gate is at 2.4 GHz
            # when real work arrives.
            zt = xp.tile([P, 640], BF16, tag="z")
            nc.gpsimd.memset(zt[:], 0.0)
            pz = pgu.tile([P, 512], F32, tag="ps", name="warm")
            for _ in range(14):
                nc.tensor.matmul(pz[:], zt[:, 0:128], zt[:, 128:640],
                                 start=True, stop=True)

            xa = xp.tile([P, DO, s0], BF16, tag="xa")
            xb = xp.tile([P, DO, s1], BF16, tag="xb")
            xseg = [xa, xb]
            gus, dts = {}, {}

            def load_gu(k, blocks):
                for b, (sfo, nfo) in enumerate(blocks):
                    g = wp.tile([P, nfo * 2 * DO, P], BF16, tag="gu",
                                name=f"gu{k}_{b}")
                    nc.sync.dma_start(
                        g[:], gus_d[k][:, sfo * 2 * DO:(sfo + nfo) * 2 * DO])
                    gus[(k, b)] = g

            def load_dw(k):
                for db in range(DO // ND):
                    dt_ = dp.tile([P, FO, ND * P], BF16, tag="dt",
                                  name=f"d{k}_{db}")
                    nc.sync.dma_start(
                        dt_[:], dws_d[k][:, db * FO:(db + 1) * FO])
                    dts[(k, db)] = dt_

            # ALL loads on the single sync HWDGE ring in strict consumption
            # order.  Critical prefix: gu-A block 0 (gate+up of fo0) + xA.
            g00 = wp.tile([P, 2 * DO, P], BF16, tag="gu", name="gu0_0")
            nc.sync.dma_start(g00[:], gus_d[0][:, 0:2 * DO])
            gus[(0, 0)] = g00
            nc.sync.dma_start(xa[:], xts_d[0][:])
            for b, (sfo, nfo) in enumerate(BLOCKS_A):
                if b == 0:
                    continue
                g = wp.tile([P, nfo * 2 * DO, P], BF16, tag="gu",
                            name=f"gu0_{b}")
                nc.sync.dma_start(
                    g[:], gus_d[0][:, sfo * 2 * DO:(sfo + nfo) * 2 * DO])
                gus[(0, b)] = g
            load_dw(0)
            nc.sync.dma_start(xb[:], xts_d[1][:])
            load_gu(1, BLOCKS_B)
            load_dw(1)

            hs = hp.tile([P, FO, C], BF16, tag="h")

            for si in range(2):
                k = si
                slen = seg_len[si]
                off = 0 if si == 0 else s0
                csl = slice(off, off + slen)
                xs = xseg[si]
                blocks = BLOCKS_A if si == 0 else BLOCKS_B
                fo2blk = {}
                for b, (sfo, nfo) in enumerate(blocks):
                    for fo in range(sfo, sfo + nfo):
                        fo2blk[fo] = b

                # gate/up for this segment
                for fo in range(FO):
                    b = fo2blk[fo]
                    fl = fo - blocks[b][0]
                    gu4 = gus[(k, b)]
                    gsel = lambda kind, do: gu4[:, (fl * 2 + kind) * DO + do, :]
                    psg = pgu.tile([P, slen], F32, tag="ps", name=f"psg_{fo}_{si}")
                    psu = pgu.tile([P, slen], F32, tag="ps", name=f"psu_{fo}_{si}")
                    for do in range(DO):
                        nc.tensor.matmul(
                            psg[:], gsel(0, do), xs[:, do, :],
                            start=(do == 0), stop=(do == DO - 1),
                        )
                    for do in range(DO):
                        nc.tensor.matmul(
                            psu[:], gsel(1, do), xs[:, do, :],
                            start=(do == 0), stop=(do == DO - 1),
                        )
                    sg = sp.tile([P, slen], F32, tag="sg")
                    nc.scalar.activation(
                        sg[:], psg[:], mybir.ActivationFunctionType.Silu
                    )
                    nc.vector.tensor_mul(out=hs[:, fo, csl], in0=sg[:], in1=psu[:])

                # down for this segment
                for db in range(DO // ND):
                    for half in range(ND):
                        do = db * ND + half
                        dsl = slice(half * P, (half + 1) * P)
                        psy = pd.tile([P, slen], F32, tag="psy",
                                      name=f"psy_{do}_{si}")
                        for fo in range(FO):
                            nc.tensor.matmul(
                                psy[:], dts[(k, db)][:, fo, dsl], hs[:, fo, csl],
                                start=(fo == 0), stop=(fo == FO - 1),
                            )
                        yo = yp.tile([P, slen], F32, tag="yo")
                        nc.any.tensor_copy(out=yo[:], in_=psy[:])
                        nc.scalar.dma_start(yt[do * P:(do + 1) * P, csl], yo[:])
    nc.finalize()
    return nc


def _get_program(s0: int, s1: int):
    key = (s0, s1)
    if key not in _BUILD_CACHE:
        _BUILD_CACHE[key] = _build(s0, s1)
    return _BUILD_CACHE[key]


# ──────────────────────────────────────────────────────────────────────
# Host routing
# ──────────────────────────────────────────────────────────────────────

def _sigmoid(z):
    return 1.0 / (1.0 + np.exp(-z))


def _route(xf32, macro_w, micro_w):
    """Host routers in float64. Returns group index per token and per-token
    weights for the 2 experts of the selected group (float32)."""
    xf = xf32.astype(np.float64)
    ms = _sigmoid(xf @ macro_w.astype(np.float64))  # [T, G]
    g_sel = np.argmax(ms, axis=1)
    T = xf.shape[0]
    mval = ms[np.arange(T), g_sel]
    mv = mval / (mval + EPS)

    w2 = np.zeros((T, 2), np.float64)
    for g in range(NUM_GROUPS):
        idx = np.nonzero(g_sel == g)[0]
        if idx.size == 0:
            continue
        s = _sigmoid(xf[idx] @ micro_w[g].astype(np.float64))  # [n, 2]
        denom = s[:, 0] + s[:, 1] + EPS
        w2[idx, 0] = mv[idx] * s[:, 0] / denom
        w2[idx, 1] = mv[idx] * s[:, 1] / denom
    return g_sel, w2.astype(np.float32)


# ──────────────────────────────────────────────────────────────────────
# Segment-size search + bucket assignment
# ──────────────────────────────────────────────────────────────────────

def _feasible(n_e, s0, s1):
    """Can loads n_e pack into 8 buckets of s0 and 8 of s1 (each bucket a
    single expert)?  Returns per-expert (a, b) bucket counts or None."""
    cands = []
    for n in n_e:
        cc = []
        if n == 0:
            cc.append((0, 0))
        else:
            for a in range(9):
                rem = n - a * s0
                b = 0 if rem <= 0 else -(-rem // s1)
                if b <= 8:
                    cc.append((a, b))
            cc.sort()
            pruned, best_b = [], 99
            for a, b in cc:
                if b < best_b:
                    pruned.append((a, b))
                    best_b = b
            cc = pruned
        cands.append(cc)
    states = {(0, 0): []}
    for cc in cands:
        nxt = {}
        for (ua, ub), hist in states.items():
            for a, b in cc:
                na, nb = ua + a, ub + b
                if na <= 8 and nb <= 8 and (na, nb) not in nxt:
                    nxt[(na, nb)] = hist + [(a, b)]
        states = nxt
        if not states:
            return None
    return next(iter(states.values()))


def _plan(n_e):
    """Pick (s0, s1, ab) minimizing C = s0 + s1."""
    key = tuple(n_e)
    if key in _PLAN_CACHE:
        return _PLAN_CACHE[key]
    best = None
    for s0 in range(64, 513, 16):
        for s1 in range(48, s0 + 1, 16):
            ab = _feasible(n_e, s0, s1)
            if ab is None:
                continue
            c = s0 + s1
            if best is None or c < best[0] or (c == best[0] and s0 < best[1]):
                best = (c, s0, s1, ab)
    if best is None:
        raise RuntimeError(f"no feasible segment plan for loads {n_e}")
    _, s0, s1, ab = best
    plan = (s0, s1, ab)
    _PLAN_CACHE[key] = plan
    return plan


# ──────────────────────────────────────────────────────────────────────
# Entry point
# ──────────────────────────────────────────────────────────────────────

def kernel(x, macro_w, micro_w, gate_w, up_w, down_w):
    global LAST_RESULTS
    x = np.asarray(x)
    B, S, D = x.shape
    T = B * S
    xf = np.ascontiguousarray(x.reshape(T, D).astype(np.float32, copy=False))

    g_sel, w2 = _route(xf, np.asarray(macro_w), np.asarray(micro_w))
    idx_by_g = [np.nonzero(g_sel == g)[0] for g in range(NUM_GROUPS)]
    n_e = [idx_by_g[e // 2].size for e in range(NUM_EXPERTS)]

    s0, s1, ab = _plan(n_e)
    segs = [(0, s0), (s0, s1)]
    nc = _get_program(s0, s1)

    gate_b = np.asarray(gate_w, np.float32).astype(ml_dtypes.bfloat16)
    up_b = np.asarray(up_w, np.float32).astype(ml_dtypes.bfloat16)
    down_b = np.asarray(down_w, np.float32).astype(ml_dtypes.bfloat16)

    # Pre-interleave weights per expert into the device DMA layouts:
    #   gu[e]  : [P, FO, 2, DO, P]   (gate,up interleaved per f-tile)
    #   dwb[e] : [P, DO//ND, FO, ND*P]
    gu_int, dw_int = [], []
    for e in range(NUM_EXPERTS):
        g5 = gate_b[e].reshape(DO, P, FO, P).transpose(1, 2, 0, 3)
        u5 = up_b[e].reshape(DO, P, FO, P).transpose(1, 2, 0, 3)
        gu_int.append(np.ascontiguousarray(
            np.stack([g5, u5], axis=2)).reshape(P, FO * 2 * DO, P))
        dw_int.append(np.ascontiguousarray(
            down_b[e].reshape(FO, P, DO // ND, ND * P).transpose(1, 2, 0, 3)
        ).reshape(P, (DO // ND) * FO, ND * P))

    # hand out buckets: free lists of (core, seg)
    free = [[(c, si) for c in range(N_CORES)] for si in range(2)]
    jobs = {}
    for e in range(NUM_EXPERTS):
        a, b = ab[e]
        ix = idx_by_g[e // 2]
        pos = 0
        for si, cnt in ((0, a), (1, b)):
            cap = (s0, s1)[si]
            for _ in range(cnt):
                c, _si = free[si].pop(0)
                take = ix[pos:pos + cap]
                pos += cap
                jobs[(c, si)] = (e, take)
    for si in range(2):
        for c, _ in free[si]:
            jobs[(c, si)] = (0, np.empty(0, np.int64))

    xfb = xf.astype(ml_dtypes.bfloat16)
    in_maps = []
    for c in range(N_CORES):
        m = {}
        for si, (off, slen) in enumerate(segs):
            e, ix = jobs[(c, si)]
            xt = np.zeros((P, DO, slen), ml_dtypes.bfloat16)
            if ix.size:
                # [n, D] -> [D, n] -> [DO, P, n] -> [P, DO, n]
                xt[:, :, :ix.size] = (
                    xfb[ix].T.reshape(DO, P, ix.size).transpose(1, 0, 2))
            m["xta" if si == 0 else "xtb"] = xt
            m[f"gu{si}"] = gu_int[e]
            m[f"dwb{si}"] = dw_int[e]
        in_maps.append(m)

    res = run_bass_kernel_spmd(nc, in_maps, core_ids=list(range(N_CORES)))
    LAST_RESULTS = res

    y = np.zeros((T, D), np.float32)
    for c in range(N_CORES):
        ytc = res.results[c]["yt"]
        for si, (off, slen) in enumerate(segs):
            e, ix = jobs[(c, si)]
            if ix.size:
                y[ix] += w2[ix, e % 2][:, None] * ytc[:, off:off + ix.size].T
    return y.reshape(B, S, D)


# revision 18
# speedup vs baseline: 1.0291x; 1.0277x over previous
"""MoE FFN (grouped top-1 routing, SwiGLU experts) on 8 Trainium2 NeuronCores.

Strategy (expert-parallel with static 2-segment load balancing):
  - Host computes the (tiny) routers in float64: sigmoid(x @ macro_w) -> top-1
    group of 4; within the selected group both 2 experts are active
    (TOP_K == EXPERTS_PER_GROUP) with sigmoid-normalized weights.
  - Each core processes C = s0 + s1 token columns in two statically-sized
    segments.  Each segment has its own full SwiGLU weight-set input, so the
    host can assign ANY expert to any (core, segment) bucket.  A small search
    picks (s0, s1) so the 8 expert token-loads pack into the 16 buckets with
    minimal C (544 vs 608 for naive per-expert capacity on the benchmark
    routing distribution).
  - The per-token routing weight is applied on the HOST to the fp32 partial
    outputs (y[t] = w0*y_e0[t] + w1*y_e1[t]), so the device never sees it and
    x is shipped only once.
  - Weights are pre-interleaved on the host into partition-major tensors
    ([P, FO, {gate,up}, DO, P]) so every DMA block is one contiguous run per
    partition AND delivers gate+up for an f-tile together, in exact PE
    consumption order on a single HWDGE ring.  Segment A runs fully
    (gate/up then down), then segment B, halving the early weight-demand
    rate vs the ~358 GB/s per-core HBM cap.
  - Device kernel per segment: Y^T = down^T @ (silu(gate^T X^T) * (up^T X^T)),
    features on SBUF partitions, tokens on the free dim, bf16 storage/matmuls,
    fp32 PSUM accumulation.  Dummy matmuls on a zeroed tile during the
    initial DMA wait warm the PE clock-gate (HAM) to 2.4 GHz for free.
"""

import numpy as np
import ml_dtypes

import concourse.bass as bass  # noqa: F401  (bass types via bacc)
import concourse.mybir as mybir
import concourse.tile as tile
from concourse import bacc
from concourse.bass_utils import run_bass_kernel_spmd

P = 128
D_MODEL = 1024
FFN_DIM = 2048
NUM_EXPERTS = 8
NUM_GROUPS = 4
EPS = 1e-9
DO = D_MODEL // P   # 8 k-tiles over D
FO = FFN_DIM // P   # 16 f-tiles over F
ND = 2              # d-tiles per down-weight block

F32 = mybir.dt.float32
BF16 = mybir.dt.bfloat16

N_CORES = 8

# gate/up f-blocks (start_fo, n_fo): segment A leads with 1-fo blocks so the
# PE starts as soon as possible; later/segment-B blocks are wide.
BLOCKS_A = [(0, 1), (1, 1), (2, 2), (4, 2), (6, 2), (8, 4), (12, 4)]
BLOCKS_B = [(0, 4), (4, 4), (8, 4), (12, 4)]

_BUILD_CACHE: dict[tuple, object] = {}
_PLAN_CACHE: dict[tuple, tuple] = {}
LAST_RESULTS = None  # stashed BassKernelResults for test harnesses


# ──────────────────────────────────────────────────────────────────────
# Device program
# ──────────────────────────────────────────────────────────────────────

def _build(s0: int, s1: int):
    """Bass/Tile program: C=s0+s1 token columns, two segments, each with its
    own full SwiGLU weight set in host-pre-interleaved layout."""
    C = s0 + s1
    seg_len = [s0, s1]

    nc = bacc.Bacc(
        "TRN2",
        target_bir_lowering=False,
        debug=False,
        enable_asserts=False,
        num_devices=N_CORES,
    )
    xts_d = [
        nc.dram_tensor("xta", [P, DO, s0], BF16, kind="ExternalInput").ap(),
        nc.dram_tensor("xtb", [P, DO, s1], BF16, kind="ExternalInput").ap(),
    ]
    gus_d, dws_d = [], []
    for k in range(2):
        gus_d.append(nc.dram_tensor(f"gu{k}", [P, FO * 2 * DO, P], BF16,
                                    kind="ExternalInput").ap())
        dws_d.append(nc.dram_tensor(f"dwb{k}", [P, (DO // ND) * FO, ND * P],
                                    BF16, kind="ExternalInput").ap())
    yt = nc.dram_tensor("yt", [D_MODEL, C], F32, kind="ExternalOutput").ap()

    with tile.TileContext(nc) as tc:
        with (
            tc.tile_pool(name="xp", bufs=1) as xp,
            tc.tile_pool(name="wp", bufs=4) as wp,
            tc.tile_pool(name="dp", bufs=4) as dp,
            tc.tile_pool(name="hp", bufs=1) as hp,
            tc.tile_pool(name="sp", bufs=4) as sp,
            tc.tile_pool(name="yp", bufs=4) as yp,
            tc.tile_pool(name="pgu", bufs=6, space="PSUM") as pgu,
            tc.tile_pool(name="pd", bufs=2, space="PSUM") as pd,
        ):
            # HAM warm-up: dummy matmuls on a zeroed tile keep the PE busy
            # during the initial DMA wait so the clock gate is at 2.4 GHz
            # when real work arrives.
            zt = xp.tile([P, 640], BF16, tag="z")
            nc.gpsimd.memset(zt[:], 0.0)
            pz = pgu.tile([P, 512], F32, tag="ps", name="warm")
            for _ in range(14):
                nc.tensor.matmul(pz[:], zt[:, 0:128], zt[:, 128:640],
                                 start=True, stop=True)

            xa = xp.tile([P, DO, s0], BF16, tag="xa")
            xb = xp.tile([P, DO, s1], BF16, tag="xb")
            xseg = [xa, xb]
            gus, dts = {}, {}

            def load_gu(k, blocks):
                for b, (sfo, nfo) in enumerate(blocks):
                    g = wp.tile([P, nfo * 2 * DO, P], BF16, tag="gu",
                                name=f"gu{k}_{b}")
                    nc.sync.dma_start(
                        g[:], gus_d[k][:, sfo * 2 * DO:(sfo + nfo) * 2 * DO])
                    gus[(k, b)] = g

            def load_gu_blk(k, b, sfo, nfo):
                g = wp.tile([P, nfo * 2 * DO, P], BF16, tag="gu",
                            name=f"gu{k}_{b}")
                nc.sync.dma_start(
                    g[:], gus_d[k][:, sfo * 2 * DO:(sfo + nfo) * 2 * DO])
                gus[(k, b)] = g

            def load_dw(k, dbs=None):
                for db in (range(DO // ND) if dbs is None else dbs):
                    dt_ = dp.tile([P, FO, ND * P], BF16, tag="dt",
                                  name=f"d{k}_{db}")
                    nc.sync.dma_start(
                        dt_[:], dws_d[k][:, db * FO:(db + 1) * FO])
                    dts[(k, db)] = dt_

            # ALL loads on the single sync HWDGE ring in strict consumption
            # order.  Critical prefix: gu-A block 0 (gate+up of fo0) + xA.
            g00 = wp.tile([P, 2 * DO, P], BF16, tag="gu", name="gu0_0")
            nc.sync.dma_start(g00[:], gus_d[0][:, 0:2 * DO])
            gus[(0, 0)] = g00
            nc.sync.dma_start(xa[:], xts_d[0][:])
            for b, (sfo, nfo) in enumerate(BLOCKS_A):
                if b == 0:
                    continue
                g = wp.tile([P, nfo * 2 * DO, P], BF16, tag="gu",
                            name=f"gu0_{b}")
                nc.sync.dma_start(
                    g[:], gus_d[0][:, sfo * 2 * DO:(sfo + nfo) * 2 * DO])
                gus[(0, b)] = g
            # dwA in two halves with segment B's first gate/up block between
            # them: B's block 0 then has ~20us of arrival margin before the
            # B phase starts even on a slow-DMA device, while dwA's own
            # deadlines keep >5us slack.
            load_dw(0, [0, 1])
            load_gu_blk(1, 0, *BLOCKS_B[0])
            load_dw(0, [2, 3])
            nc.sync.dma_start(xb[:], xts_d[1][:])
            for b, (sfo, nfo) in enumerate(BLOCKS_B):
                if b > 0:
                    load_gu_blk(1, b, sfo, nfo)
            load_dw(1)

            hs = hp.tile([P, FO, C], BF16, tag="h")

            for si in range(2):
                k = si
                slen = seg_len[si]
                off = 0 if si == 0 else s0
                csl = slice(off, off + slen)
                xs = xseg[si]
                blocks = BLOCKS_A if si == 0 else BLOCKS_B
                fo2blk = {}
                for b, (sfo, nfo) in enumerate(blocks):
                    for fo in range(sfo, sfo + nfo):
                        fo2blk[fo] = b

                # gate/up for this segment
                for fo in range(FO):
                    b = fo2blk[fo]
                    fl = fo - blocks[b][0]
                    gu4 = gus[(k, b)]
                    gsel = lambda kind, do: gu4[:, (fl * 2 + kind) * DO + do, :]
                    psg = pgu.tile([P, slen], F32, tag="ps", name=f"psg_{fo}_{si}")
                    psu = pgu.tile([P, slen], F32, tag="ps", name=f"psu_{fo}_{si}")
                    for do in range(DO):
                        nc.tensor.matmul(
                            psg[:], gsel(0, do), xs[:, do, :],
                            start=(do == 0), stop=(do == DO - 1),
                        )
                    for do in range(DO):
                        nc.tensor.matmul(
                            psu[:], gsel(1, do), xs[:, do, :],
                            start=(do == 0), stop=(do == DO - 1),
                        )
                    sg = sp.tile([P, slen], F32, tag="sg")
                    nc.scalar.activation(
                        sg[:], psg[:], mybir.ActivationFunctionType.Silu
                    )
                    nc.vector.tensor_mul(out=hs[:, fo, csl], in0=sg[:], in1=psu[:])

                # down for this segment
                for db in range(DO // ND):
                    for half in range(ND):
                        do = db * ND + half
                        dsl = slice(half * P, (half + 1) * P)
                        psy = pd.tile([P, slen], F32, tag="psy",
                                      name=f"psy_{do}_{si}")
                        for fo in range(FO):
                            nc.tensor.matmul(
                                psy[:], dts[(k, db)][:, fo, dsl], hs[:, fo, csl],
                                start=(fo == 0), stop=(fo == FO - 1),
                            )
                        yo = yp.tile([P, slen], F32, tag="yo")
                        nc.any.tensor_copy(out=yo[:], in_=psy[:])
                        nc.scalar.dma_start(yt[do * P:(do + 1) * P, csl], yo[:])
    nc.finalize()
    return nc


def _get_program(s0: int, s1: int):
    key = (s0, s1)
    if key not in _BUILD_CACHE:
        _BUILD_CACHE[key] = _build(s0, s1)
    return _BUILD_CACHE[key]


# ──────────────────────────────────────────────────────────────────────
# Host routing
# ──────────────────────────────────────────────────────────────────────

def _sigmoid(z):
    return 1.0 / (1.0 + np.exp(-z))


def _route(xf32, macro_w, micro_w):
    """Host routers in float64. Returns group index per token and per-token
    weights for the 2 experts of the selected group (float32)."""
    xf = xf32.astype(np.float64)
    ms = _sigmoid(xf @ macro_w.astype(np.float64))  # [T, G]
    g_sel = np.argmax(ms, axis=1)
    T = xf.shape[0]
    mval = ms[np.arange(T), g_sel]
    mv = mval / (mval + EPS)

    w2 = np.zeros((T, 2), np.float64)
    for g in range(NUM_GROUPS):
        idx = np.nonzero(g_sel == g)[0]
        if idx.size == 0:
            continue
        s = _sigmoid(xf[idx] @ micro_w[g].astype(np.float64))  # [n, 2]
        denom = s[:, 0] + s[:, 1] + EPS
        w2[idx, 0] = mv[idx] * s[:, 0] / denom
        w2[idx, 1] = mv[idx] * s[:, 1] / denom
    return g_sel, w2.astype(np.float32)


# ──────────────────────────────────────────────────────────────────────
# Segment-size search + bucket assignment
# ──────────────────────────────────────────────────────────────────────

def _feasible(n_e, s0, s1):
    """Can loads n_e pack into 8 buckets of s0 and 8 of s1 (each bucket a
    single expert)?  Returns per-expert (a, b) bucket counts or None."""
    cands = []
    for n in n_e:
        cc = []
        if n == 0:
            cc.append((0, 0))
        else:
            for a in range(9):
                rem = n - a * s0
                b = 0 if rem <= 0 else -(-rem // s1)
                if b <= 8:
                    cc.append((a, b))
            cc.sort()
            pruned, best_b = [], 99
            for a, b in cc:
                if b < best_b:
                    pruned.append((a, b))
                    best_b = b
            cc = pruned
        cands.append(cc)
    states = {(0, 0): []}
    for cc in cands:
        nxt = {}
        for (ua, ub), hist in states.items():
            for a, b in cc:
                na, nb = ua + a, ub + b
                if na <= 8 and nb <= 8 and (na, nb) not in nxt:
                    nxt[(na, nb)] = hist + [(a, b)]
        states = nxt
        if not states:
            return None
    return next(iter(states.values()))


def _plan(n_e):
    """Pick (s0, s1, ab) minimizing C = s0 + s1."""
    key = tuple(n_e)
    if key in _PLAN_CACHE:
        return _PLAN_CACHE[key]
    best = None
    for s0 in range(64, 513, 16):
        for s1 in range(48, s0 + 1, 16):
            ab = _feasible(n_e, s0, s1)
            if ab is None:
                continue
            c = s0 + s1
            if best is None or c < best[0] or (c == best[0] and s0 < best[1]):
                best = (c, s0, s1, ab)
    if best is None:
        raise RuntimeError(f"no feasible segment plan for loads {n_e}")
    _, s0, s1, ab = best
    plan = (s0, s1, ab)
    _PLAN_CACHE[key] = plan
    return plan


# ──────────────────────────────────────────────────────────────────────
# Entry point
# ──────────────────────────────────────────────────────────────────────

def kernel(x, macro_w, micro_w, gate_w, up_w, down_w):
    global LAST_RESULTS
    x = np.asarray(x)
    B, S, D = x.shape
    T = B * S
    xf = np.ascontiguousarray(x.reshape(T, D).astype(np.float32, copy=False))

    g_sel, w2 = _route(xf, np.asarray(macro_w), np.asarray(micro_w))
    idx_by_g = [np.nonzero(g_sel == g)[0] for g in range(NUM_GROUPS)]
    n_e = [idx_by_g[e // 2].size for e in range(NUM_EXPERTS)]

    s0, s1, ab = _plan(n_e)
    segs = [(0, s0), (s0, s1)]
    nc = _get_program(s0, s1)

    gate_b = np.asarray(gate_w, np.float32).astype(ml_dtypes.bfloat16)
    up_b = np.asarray(up_w, np.float32).astype(ml_dtypes.bfloat16)
    down_b = np.asarray(down_w, np.float32).astype(ml_dtypes.bfloat16)

    # Pre-interleave weights per expert into the device DMA layouts:
    #   gu[e]  : [P, FO, 2, DO, P]   (gate,up interleaved per f-tile)
    #   dwb[e] : [P, DO//ND, FO, ND*P]
    gu_int, dw_int = [], []
    for e in range(NUM_EXPERTS):
        g5 = gate_b[e].reshape(DO, P, FO, P).transpose(1, 2, 0, 3)
        u5 = up_b[e].reshape(DO, P, FO, P).transpose(1, 2, 0, 3)
        gu_int.append(np.ascontiguousarray(
            np.stack([g5, u5], axis=2)).reshape(P, FO * 2 * DO, P))
        dw_int.append(np.ascontiguousarray(
            down_b[e].reshape(FO, P, DO // ND, ND * P).transpose(1, 2, 0, 3)
        ).reshape(P, (DO // ND) * FO, ND * P))

    # hand out buckets: free lists of (core, seg)
    free = [[(c, si) for c in range(N_CORES)] for si in range(2)]
    jobs = {}
    for e in range(NUM_EXPERTS):
        a, b = ab[e]
        ix = idx_by_g[e // 2]
        pos = 0
        for si, cnt in ((0, a), (1, b)):
            cap = (s0, s1)[si]
            for _ in range(cnt):
                c, _si = free[si].pop(0)
                take = ix[pos:pos + cap]
                pos += cap
                jobs[(c, si)] = (e, take)
    for si in range(2):
        for c, _ in free[si]:
            jobs[(c, si)] = (0, np.empty(0, np.int64))

    xfb = xf.astype(ml_dtypes.bfloat16)
    in_maps = []
    for c in range(N_CORES):
        m = {}
        for si, (off, slen) in enumerate(segs):
            e, ix = jobs[(c, si)]
            xt = np.zeros((P, DO, slen), ml_dtypes.bfloat16)
            if ix.size:
                # [n, D] -> [D, n] -> [DO, P, n] -> [P, DO, n]
                xt[:, :, :ix.size] = (
                    xfb[ix].T.reshape(DO, P, ix.size).transpose(1, 0, 2))
            m["xta" if si == 0 else "xtb"] = xt
            m[f"gu{si}"] = gu_int[e]
            m[f"dwb{si}"] = dw_int[e]
        in_maps.append(m)

    res = run_bass_kernel_spmd(nc, in_maps, core_ids=list(range(N_CORES)))
    LAST_RESULTS = res

    y = np.zeros((T, D), np.float32)
    for c in range(N_CORES):
        ytc = res.results[c]["yt"]
        for si, (off, slen) in enumerate(segs):
            e, ix = jobs[(c, si)]
            if ix.size:
                y[ix] += w2[ix, e % 2][:, None] * ytc[:, off:off + ix.size].T
    return y.reshape(B, S, D)


# revision 19
# speedup vs baseline: 1.0643x; 1.0342x over previous
"""MoE FFN (grouped top-1 routing, SwiGLU experts) on 8 Trainium2 NeuronCores.

Strategy (expert-parallel with static 2-segment load balancing):
  - Host computes the (tiny) routers in float64: sigmoid(x @ macro_w) -> top-1
    group of 4; within the selected group both 2 experts are active
    (TOP_K == EXPERTS_PER_GROUP) with sigmoid-normalized weights.
  - Each core processes C = s0 + s1 token columns in two statically-sized
    segments.  Each segment has its own full SwiGLU weight-set input, so the
    host can assign ANY expert to any (core, segment) bucket.  A small search
    picks (s0, s1) so the 8 expert token-loads pack into the 16 buckets with
    minimal C (544 vs 608 for naive per-expert capacity on the benchmark
    routing distribution).
  - The per-token routing weight is applied on the HOST to the fp32 partial
    outputs (y[t] = w0*y_e0[t] + w1*y_e1[t]), so the device never sees it and
    x is shipped only once.
  - Weights are pre-interleaved on the host into partition-major tensors
    ([P, FO, {gate,up}, DO, P]) so every DMA block is one contiguous run per
    partition AND delivers gate+up for an f-tile together, in exact PE
    consumption order on a single HWDGE ring.  Segment A runs fully
    (gate/up then down), then segment B, halving the early weight-demand
    rate vs the ~358 GB/s per-core HBM cap.
  - Device kernel per segment: Y^T = down^T @ (silu(gate^T X^T) * (up^T X^T)),
    features on SBUF partitions, tokens on the free dim, bf16 storage/matmuls,
    fp32 PSUM accumulation.  Dummy matmuls on a zeroed tile during the
    initial DMA wait warm the PE clock-gate (HAM) to 2.4 GHz for free.
"""

import numpy as np
import ml_dtypes

import concourse.bass as bass  # noqa: F401  (bass types via bacc)
import concourse.mybir as mybir
import concourse.tile as tile
from concourse import bacc
from concourse.bass_utils import run_bass_kernel_spmd

P = 128
D_MODEL = 1024
FFN_DIM = 2048
NUM_EXPERTS = 8
NUM_GROUPS = 4
EPS = 1e-9
DO = D_MODEL // P   # 8 k-tiles over D
FO = FFN_DIM // P   # 16 f-tiles over F
ND = 2              # d-tiles per down-weight block

F32 = mybir.dt.float32
BF16 = mybir.dt.bfloat16

N_CORES = 8

# gate/up f-blocks (start_fo, n_fo): segment A leads with 1-fo blocks so the
# PE starts as soon as possible; later/segment-B blocks are wide.
BLOCKS_A = [(0, 1), (1, 1), (2, 2), (4, 2), (6, 2), (8, 4), (12, 4)]
BLOCKS_B = [(0, 1), (1, 1), (2, 2), (4, 4), (8, 4), (12, 4)]

_BUILD_CACHE: dict[tuple, object] = {}
_PLAN_CACHE: dict[tuple, tuple] = {}
LAST_RESULTS = None  # stashed BassKernelResults for test harnesses


# ──────────────────────────────────────────────────────────────────────
# Device program
# ──────────────────────────────────────────────────────────────────────

def _build(s0: int, s1: int):
    """Bass/Tile program: C=s0+s1 token columns, two segments, each with its
    own full SwiGLU weight set in host-pre-interleaved layout."""
    C = s0 + s1
    seg_len = [s0, s1]

    nc = bacc.Bacc(
        "TRN2",
        target_bir_lowering=False,
        debug=False,
        enable_asserts=False,
        num_devices=N_CORES,
    )
    xts_d = [
        nc.dram_tensor("xta", [P, DO, s0], BF16, kind="ExternalInput").ap(),
        nc.dram_tensor("xtb", [P, DO, s1], BF16, kind="ExternalInput").ap(),
    ]
    gus_d, dws_d = [], []
    for k in range(2):
        gus_d.append(nc.dram_tensor(f"gu{k}", [P, FO * 2 * DO, P], BF16,
                                    kind="ExternalInput").ap())
        dws_d.append(nc.dram_tensor(f"dwb{k}", [P, (DO // ND) * FO, ND * P],
                                    BF16, kind="ExternalInput").ap())
    yt = nc.dram_tensor("yt", [D_MODEL, C], F32, kind="ExternalOutput").ap()

    with tile.TileContext(nc) as tc:
        with (
            tc.tile_pool(name="xp", bufs=1) as xp,
            tc.tile_pool(name="wp", bufs=4) as wp,
            tc.tile_pool(name="dp", bufs=4) as dp,
            tc.tile_pool(name="hp", bufs=1) as hp,
            tc.tile_pool(name="sp", bufs=4) as sp,
            tc.tile_pool(name="yp", bufs=4) as yp,
            tc.tile_pool(name="pgu", bufs=6, space="PSUM") as pgu,
            tc.tile_pool(name="pd", bufs=2, space="PSUM") as pd,
        ):
            # HAM warm-up: dummy matmuls on a zeroed tile keep the PE busy
            # during the initial DMA wait so the clock gate is at 2.4 GHz
            # when real work arrives.
            zt = xp.tile([P, 640], BF16, tag="z")
            nc.gpsimd.memset(zt[:], 0.0)
            pz = pgu.tile([P, 512], F32, tag="ps", name="warm")
            for _ in range(14):
                nc.tensor.matmul(pz[:], zt[:, 0:128], zt[:, 128:640],
                                 start=True, stop=True)

            xa = xp.tile([P, DO, s0], BF16, tag="xa")
            xb = xp.tile([P, DO, s1], BF16, tag="xb")
            xseg = [xa, xb]
            gus, dts = {}, {}

            def load_gu(k, blocks):
                for b, (sfo, nfo) in enumerate(blocks):
                    g = wp.tile([P, nfo * 2 * DO, P], BF16, tag="gu",
                                name=f"gu{k}_{b}")
                    nc.sync.dma_start(
                        g[:], gus_d[k][:, sfo * 2 * DO:(sfo + nfo) * 2 * DO])
                    gus[(k, b)] = g

            def load_gu_blk(k, b, sfo, nfo):
                g = wp.tile([P, nfo * 2 * DO, P], BF16, tag="gu",
                            name=f"gu{k}_{b}")
                nc.sync.dma_start(
                    g[:], gus_d[k][:, sfo * 2 * DO:(sfo + nfo) * 2 * DO])
                gus[(k, b)] = g

            def load_dw(k, dbs=None):
                for db in (range(DO // ND) if dbs is None else dbs):
                    dt_ = dp.tile([P, FO, ND * P], BF16, tag="dt",
                                  name=f"d{k}_{db}")
                    nc.sync.dma_start(
                        dt_[:], dws_d[k][:, db * FO:(db + 1) * FO])
                    dts[(k, db)] = dt_

            # ALL loads on the single sync HWDGE ring in strict consumption
            # order.  Critical prefix: gu-A block 0 (gate+up of fo0) + xA.
            g00 = wp.tile([P, 2 * DO, P], BF16, tag="gu", name="gu0_0")
            nc.sync.dma_start(g00[:], gus_d[0][:, 0:2 * DO])
            gus[(0, 0)] = g00
            nc.sync.dma_start(xa[:], xts_d[0][:])
            for b, (sfo, nfo) in enumerate(BLOCKS_A):
                if b == 0:
                    continue
                g = wp.tile([P, nfo * 2 * DO, P], BF16, tag="gu",
                            name=f"gu0_{b}")
                nc.sync.dma_start(
                    g[:], gus_d[0][:, sfo * 2 * DO:(sfo + nfo) * 2 * DO])
                gus[(0, b)] = g
            # dwA in two halves with segment B's first gate/up block between
            # them: B's block 0 then has ~20us of arrival margin before the
            # B phase starts even on a slow-DMA device, while dwA's own
            # deadlines keep >5us slack.
            load_dw(0, [0, 1])
            load_gu_blk(1, 0, *BLOCKS_B[0])
            load_dw(0, [2, 3])
            nc.sync.dma_start(xb[:], xts_d[1][:])
            for b, (sfo, nfo) in enumerate(BLOCKS_B):
                if b > 0:
                    load_gu_blk(1, b, sfo, nfo)
            load_dw(1)

            hs = hp.tile([P, FO, C], BF16, tag="h")

            for si in range(2):
                k = si
                slen = seg_len[si]
                off = 0 if si == 0 else s0
                csl = slice(off, off + slen)
                xs = xseg[si]
                blocks = BLOCKS_A if si == 0 else BLOCKS_B
                fo2blk = {}
                for b, (sfo, nfo) in enumerate(blocks):
                    for fo in range(sfo, sfo + nfo):
                        fo2blk[fo] = b

                # gate/up for this segment
                for fo in range(FO):
                    b = fo2blk[fo]
                    fl = fo - blocks[b][0]
                    gu4 = gus[(k, b)]
                    gsel = lambda kind, do: gu4[:, (fl * 2 + kind) * DO + do, :]
                    psg = pgu.tile([P, slen], F32, tag="ps", name=f"psg_{fo}_{si}")
                    psu = pgu.tile([P, slen], F32, tag="ps", name=f"psu_{fo}_{si}")
                    for do in range(DO):
                        nc.tensor.matmul(
                            psg[:], gsel(0, do), xs[:, do, :],
                            start=(do == 0), stop=(do == DO - 1),
                        )
                    for do in range(DO):
                        nc.tensor.matmul(
                            psu[:], gsel(1, do), xs[:, do, :],
                            start=(do == 0), stop=(do == DO - 1),
                        )
                    sg = sp.tile([P, slen], F32, tag="sg")
                    nc.scalar.activation(
                        sg[:], psg[:], mybir.ActivationFunctionType.Silu
                    )
                    nc.vector.tensor_mul(out=hs[:, fo, csl], in0=sg[:], in1=psu[:])

                # down for this segment
                for db in range(DO // ND):
                    for half in range(ND):
                        do = db * ND + half
                        dsl = slice(half * P, (half + 1) * P)
                        psy = pd.tile([P, slen], F32, tag="psy",
                                      name=f"psy_{do}_{si}")
                        for fo in range(FO):
                            nc.tensor.matmul(
                                psy[:], dts[(k, db)][:, fo, dsl], hs[:, fo, csl],
                                start=(fo == 0), stop=(fo == FO - 1),
                            )
                        yo = yp.tile([P, slen], F32, tag="yo")
                        nc.any.tensor_copy(out=yo[:], in_=psy[:])
                        nc.scalar.dma_start(yt[do * P:(do + 1) * P, csl], yo[:])
    nc.finalize()
    return nc


def _get_program(s0: int, s1: int):
    key = (s0, s1)
    if key not in _BUILD_CACHE:
        _BUILD_CACHE[key] = _build(s0, s1)
    return _BUILD_CACHE[key]


# ──────────────────────────────────────────────────────────────────────
# Host routing
# ──────────────────────────────────────────────────────────────────────

def _sigmoid(z):
    return 1.0 / (1.0 + np.exp(-z))


def _route(xf32, macro_w, micro_w):
    """Host routers in float64. Returns group index per token and per-token
    weights for the 2 experts of the selected group (float32)."""
    xf = xf32.astype(np.float64)
    ms = _sigmoid(xf @ macro_w.astype(np.float64))  # [T, G]
    g_sel = np.argmax(ms, axis=1)
    T = xf.shape[0]
    mval = ms[np.arange(T), g_sel]
    mv = mval / (mval + EPS)

    w2 = np.zeros((T, 2), np.float64)
    for g in range(NUM_GROUPS):
        idx = np.nonzero(g_sel == g)[0]
        if idx.size == 0:
            continue
        s = _sigmoid(xf[idx] @ micro_w[g].astype(np.float64))  # [n, 2]
        denom = s[:, 0] + s[:, 1] + EPS
        w2[idx, 0] = mv[idx] * s[:, 0] / denom
        w2[idx, 1] = mv[idx] * s[:, 1] / denom
    return g_sel, w2.astype(np.float32)


# ──────────────────────────────────────────────────────────────────────
# Segment-size search + bucket assignment
# ──────────────────────────────────────────────────────────────────────

def _feasible(n_e, s0, s1):
    """Can loads n_e pack into 8 buckets of s0 and 8 of s1 (each bucket a
    single expert)?  Returns per-expert (a, b) bucket counts or None."""
    cands = []
    for n in n_e:
        cc = []
        if n == 0:
            cc.append((0, 0))
        else:
            for a in range(9):
                rem = n - a * s0
                b = 0 if rem <= 0 else -(-rem // s1)
                if b <= 8:
                    cc.append((a, b))
            cc.sort()
            pruned, best_b = [], 99
            for a, b in cc:
                if b < best_b:
                    pruned.append((a, b))
                    best_b = b
            cc = pruned
        cands.append(cc)
    states = {(0, 0): []}
    for cc in cands:
        nxt = {}
        for (ua, ub), hist in states.items():
            for a, b in cc:
                na, nb = ua + a, ub + b
                if na <= 8 and nb <= 8 and (na, nb) not in nxt:
                    nxt[(na, nb)] = hist + [(a, b)]
        states = nxt
        if not states:
            return None
    return next(iter(states.values()))


def _plan(n_e):
    """Pick (s0, s1, ab) minimizing C = s0 + s1."""
    key = tuple(n_e)
    if key in _PLAN_CACHE:
        return _PLAN_CACHE[key]
    best = None
    for s0 in range(64, 513, 16):
        for s1 in range(48, s0 + 1, 16):
            ab = _feasible(n_e, s0, s1)
            if ab is None:
                continue
            c = s0 + s1
            if best is None or c < best[0] or (c == best[0] and s0 < best[1]):
                best = (c, s0, s1, ab)
    if best is None:
        raise RuntimeError(f"no feasible segment plan for loads {n_e}")
    _, s0, s1, ab = best
    plan = (s0, s1, ab)
    _PLAN_CACHE[key] = plan
    return plan


# ──────────────────────────────────────────────────────────────────────
# Entry point
# ──────────────────────────────────────────────────────────────────────

def kernel(x, macro_w, micro_w, gate_w, up_w, down_w):
    global LAST_RESULTS
    x = np.asarray(x)
    B, S, D = x.shape
    T = B * S
    xf = np.ascontiguousarray(x.reshape(T, D).astype(np.float32, copy=False))

    g_sel, w2 = _route(xf, np.asarray(macro_w), np.asarray(micro_w))
    idx_by_g = [np.nonzero(g_sel == g)[0] for g in range(NUM_GROUPS)]
    n_e = [idx_by_g[e // 2].size for e in range(NUM_EXPERTS)]

    s0, s1, ab = _plan(n_e)
    segs = [(0, s0), (s0, s1)]
    nc = _get_program(s0, s1)

    gate_b = np.asarray(gate_w, np.float32).astype(ml_dtypes.bfloat16)
    up_b = np.asarray(up_w, np.float32).astype(ml_dtypes.bfloat16)
    down_b = np.asarray(down_w, np.float32).astype(ml_dtypes.bfloat16)

    # Pre-interleave weights per expert into the device DMA layouts:
    #   gu[e]  : [P, FO, 2, DO, P]   (gate,up interleaved per f-tile)
    #   dwb[e] : [P, DO//ND, FO, ND*P]
    gu_int, dw_int = [], []
    for e in range(NUM_EXPERTS):
        g5 = gate_b[e].reshape(DO, P, FO, P).transpose(1, 2, 0, 3)
        u5 = up_b[e].reshape(DO, P, FO, P).transpose(1, 2, 0, 3)
        gu_int.append(np.ascontiguousarray(
            np.stack([g5, u5], axis=2)).reshape(P, FO * 2 * DO, P))
        dw_int.append(np.ascontiguousarray(
            down_b[e].reshape(FO, P, DO // ND, ND * P).transpose(1, 2, 0, 3)
        ).reshape(P, (DO // ND) * FO, ND * P))

    # hand out buckets: free lists of (core, seg)
    free = [[(c, si) for c in range(N_CORES)] for si in range(2)]
    jobs = {}
    for e in range(NUM_EXPERTS):
        a, b = ab[e]
        ix = idx_by_g[e // 2]
        pos = 0
        for si, cnt in ((0, a), (1, b)):
            cap = (s0, s1)[si]
            for _ in range(cnt):
                c, _si = free[si].pop(0)
                take = ix[pos:pos + cap]
                pos += cap
                jobs[(c, si)] = (e, take)
    for si in range(2):
        for c, _ in free[si]:
            jobs[(c, si)] = (0, np.empty(0, np.int64))

    xfb = xf.astype(ml_dtypes.bfloat16)
    in_maps = []
    for c in range(N_CORES):
        m = {}
        for si, (off, slen) in enumerate(segs):
            e, ix = jobs[(c, si)]
            xt = np.zeros((P, DO, slen), ml_dtypes.bfloat16)
            if ix.size:
                # [n, D] -> [D, n] -> [DO, P, n] -> [P, DO, n]
                xt[:, :, :ix.size] = (
                    xfb[ix].T.reshape(DO, P, ix.size).transpose(1, 0, 2))
            m["xta" if si == 0 else "xtb"] = xt
            m[f"gu{si}"] = gu_int[e]
            m[f"dwb{si}"] = dw_int[e]
        in_maps.append(m)

    res = run_bass_kernel_spmd(nc, in_maps, core_ids=list(range(N_CORES)))
    LAST_RESULTS = res

    y = np.zeros((T, D), np.float32)
    for c in range(N_CORES):
        ytc = res.results[c]["yt"]
        for si, (off, slen) in enumerate(segs):
            e, ix = jobs[(c, si)]
            if ix.size:
                y[ix] += w2[ix, e % 2][:, None] * ytc[:, off:off + ix.size].T
    return y.reshape(B, S, D)
